# revision 1
# baseline (speedup 1.0000x reference)
"""Trainium2 Bass kernel for nn_LocalEnergyOpt (molecular-mechanics local energy).

Per batch sample (B=128): features[:, :, 5] packs coords [4096, 3]; col 6 bonds
(i,j,t)x4095; col 7 angles (i,j,k,t)x4094; col 8 torsions (i,j,k,l,t)x4093.
  e_bond = opt[0] * sum k_t (|ci-cj| - r0_t)^2
  e_ang  = opt[1] * sum k_t (theta - th0_t)^2, theta = arccos(clip(cos))
  e_tor  = opt[2] * sum k_t (1 + cos(n_t phi - d_t)), phi = atan2(y, x)
Output [B, 3].

Sharding: pure data parallel, 16 samples per NeuronCore across 8 cores.

Device pipeline per NC (2 waves x 8 samples; GPSIMD Q7 core c handles sample
8w+c on partitions 16c..16c+15):
  stage features flat -> extract packed columns (stride-9 DVE copies) ->
  dense per-sample DRAM scratch -> read back as (a) a per-partition-replicated
  coords table for ap_gather, (b) [16, X] index blocks -> int16 wrap-layout
  index lists -> ap_gather endpoint coords + per-type params -> dedup the
  16x-replicated gather outputs via a DRAM round trip -> dense [128, 256]-col
  DVE/ACT energy pipeline -> masked tensor_tensor_reduce partials ->
  per-wave PE matmul (one-partition-per-group selector) -> [8, 3] -> scale by
  opt_pars[0:3] -> out.

Torsion angle avoids arccos/atan2 LUTs: cos(phi), sin(phi) are formed by
normalizing (x, y) = (n1.n2, (n1 x b2).n2 / |b2|), and cos(n phi - d) expands
via Chebyshev doubling/tripling + per-type (cos d, sin d) tables.
"""

import sys
import functools

import numpy as np

sys.path.insert(0, "/opt/trn_rl_repo")

from concourse import bacc, mybir  # noqa: E402
import concourse.tile as tile  # noqa: E402
from concourse.alu_op_type import AluOpType as Op  # noqa: E402

F32 = mybir.dt.float32
I16 = mybir.dt.int16
I32 = mybir.dt.int32
AF = mybir.ActivationFunctionType
AX = mybir.AxisListType

# Problem constants
N_CORES = 8
NS = 16                      # samples per NeuronCore
NB, NA, NT = 4095, 4094, 4093
NATOMS = 4096
MAXLEN = 20465
LPP = 1440                   # flat f32 per partition (multiple of 9)
FLATPAD = 128 * LPP          # 184320 >= 184185
CR = LPP // 9                # 160 col rows per partition
COLN = 128 * CR              # 20480 dense col length
EPS = 1e-8
PI = float(np.pi)

LIST = 4096                  # per-core index list length per class (padded)
GCH = 512                    # ap_gather chunk
NCHUNK = LIST // GCH
DP = LIST // 16              # 256 dense positions per partition


def build_nc():
    nc = bacc.Bacc(None, target_bir_lowering=False, debug=False)

    feat = nc.dram_tensor("features", [NS, FLATPAD], F32, kind="ExternalInput")
    bond_t = nc.dram_tensor("bond_type", [15, 2], F32, kind="ExternalInput")
    ang_t = nc.dram_tensor("angle_type", [13, 2], F32, kind="ExternalInput")
    tor_t = nc.dram_tensor("tor_type", [25, 2], F32, kind="ExternalInput")
    mult_f = nc.dram_tensor("mult_f", [1, 25], F32, kind="ExternalInput")
    opt_p = nc.dram_tensor("opt_pars", [1, 47], F32, kind="ExternalInput")
    out_d = nc.dram_tensor("out", [NS, 3], F32, kind="ExternalOutput")

    with tile.TileContext(nc) as tc:
        with (
            tc.tile_pool(name="const", bufs=1) as constp,
            tc.tile_pool(name="stage", bufs=2) as stagep,
            tc.tile_pool(name="cext", bufs=2) as cextp,
            tc.tile_pool(name="table", bufs=1) as tablep,
            tc.tile_pool(name="idxraw", bufs=1) as idxrawp,
            tc.tile_pool(name="idx16", bufs=1) as idx16p,
            tc.tile_pool(name="gath", bufs=2) as gathp,
            tc.tile_pool(name="dense", bufs=5) as densep,
            tc.tile_pool(name="work", bufs=1) as workp,
            tc.tile_pool(name="accp", bufs=1) as accp,
            tc.tile_pool(name="psum", bufs=1, space="PSUM") as psump,
            tc.tile_pool(name="dram", bufs=2, space="DRAM") as dramp,
            tc.tile_pool(name="dramded", bufs=6, space="DRAM") as dedp,
        ):
            # ---------------- constants (bundled into one [128, *] tile) ----
            # layout: btab[0:30] atab[30:56] ttab[56:156] maskB[156:412]
            #         maskA[412:668] maskT[668:924] blk[924:932]
            cst = constp.tile([128, 932], F32)
            btab = cst[:, 0:30]
            atab = cst[:, 30:56]
            ttab = cst[:, 56:156]
            mB = cst[:, 156:156 + DP]
            mA = cst[:, 412:412 + DP]
            mT = cst[:, 668:668 + DP]
            blk = cst[:, 924:932]
            fwork = constp.tile([128, 478], F32)
            cb = fwork[:, 258:262]
            nc.vector.memset(cb[:, 0:1], EPS)
            nc.vector.memset(cb[:, 1:2], PI / 2.0)
            nc.vector.memset(cb[:, 2:3], 1e-30)
            nc.vector.memset(cb[:, 3:4], -1.0)
            b_eps = cb[:, 0:1]
            b_pi2 = cb[:, 1:2]
            b_tiny = cb[:, 2:3]
            s_neg1 = cb[:, 3:4]

            nc.sync.dma_start(
                out=btab,
                in_=bond_t.ap().rearrange("a b -> (a b)")[None, :].to_broadcast([128, 30]),
            )
            nc.sync.dma_start(
                out=atab,
                in_=ang_t.ap().rearrange("a b -> (a b)")[None, :].to_broadcast([128, 26]),
            )
            # torsion derived table (k, cos d, sin d, n) x 25 on one partition
            onep = fwork[0:1, 262:462]
            traw = onep[:, 0:50]
            mraw = onep[:, 50:75]
            t4 = onep[:, 75:175]
            nc.sync.dma_start(out=traw, in_=tor_t.ap().rearrange("a b -> (a b)")[None, :])
            nc.sync.dma_start(out=mraw, in_=mult_f.ap())
            t4v = t4.rearrange("p (n d) -> p n d", d=4)
            trv = traw.rearrange("p (n d) -> p n d", d=2)
            nc.vector.tensor_copy(out=t4v[:, :, 0], in_=trv[:, :, 0])                # k
            # cos d = sin(pi/2 - d); d in [0, 3.15) keeps the arg in [-pi, pi]
            nc.scalar.activation(t4v[:, :, 1], trv[:, :, 1], AF.Sin,
                                 bias=b_pi2[0:1, :], scale=s_neg1[0:1, :])
            nc.scalar.activation(t4v[:, :, 2], trv[:, :, 1], AF.Sin)                 # sin d
            nc.vector.tensor_copy(out=t4v[:, :, 3], in_=mraw)                        # n
            t4_dram = dramp.tile([1, 100], F32)
            nc.sync.dma_start(out=t4_dram[:], in_=t4)
            nc.sync.dma_start(out=ttab, in_=t4_dram[:].to_broadcast([128, 100]))

            # masks: invalid tail list positions live on partitions p=15 (mod 16)
            # (engine ops must start at partition 0 -> build via iota+compare)
            iwork = constp.tile([128, 268], I32)
            pidx = iwork[:, 0:1]
            colx = iwork[:, 1:257]
            and15 = iwork[:, 257:258]
            r15i = iwork[:, 258:259]
            blki = iwork[:, 259:267]
            pdiv = iwork[:, 267:268]
            row15 = fwork[:, 0:1]
            tailf = fwork[:, 1:257]
            nc.gpsimd.iota(pidx, pattern=[[1, 1]], base=0, channel_multiplier=1)
            nc.gpsimd.iota(colx, pattern=[[1, 256]], base=0, channel_multiplier=0)
            nc.vector.tensor_scalar(out=and15, in0=pidx, scalar1=15, scalar2=None,
                                    op0=Op.bitwise_and)
            nc.vector.tensor_scalar(out=r15i, in0=and15, scalar1=15, scalar2=None,
                                    op0=Op.is_equal)
            nc.vector.tensor_copy(out=row15, in_=r15i)
            for msk, ntail in ((mB, 1), (mA, 2), (mT, 3)):
                nc.vector.tensor_scalar(out=tailf, in0=colx, scalar1=DP - ntail,
                                        scalar2=None, op0=Op.is_ge)
                nc.vector.tensor_tensor(out=msk, in0=tailf,
                                        in1=row15.to_broadcast([128, DP]), op=Op.mult)
                nc.vector.tensor_scalar(out=msk, in0=msk, scalar1=-1.0, scalar2=1.0,
                                        op0=Op.mult, op1=Op.add)
            # selector: blk[p, c] = 1 iff p//16 == c -> PE sums each 16-part
            # group (the dedup slices are disjoint partials) into PSUM row c
            nc.vector.tensor_scalar(out=pdiv, in0=pidx, scalar1=4, scalar2=None,
                                    op0=Op.arith_shift_right)
            nc.gpsimd.iota(blki, pattern=[[1, 8]], base=0, channel_multiplier=0)
            nc.vector.tensor_tensor(out=blki, in0=pdiv.to_broadcast([128, 8]),
                                    in1=blki, op=Op.is_equal)
            nc.vector.tensor_copy(out=blk, in_=blki)

            accb = accp.tile([128, DP + 6 + 8], F32)
            scr = accb[:, 0:DP]            # TTR mandatory elementwise out
            acc6 = accb[:, DP:DP + 6]
            otmp = accb[0:8, DP + 6:DP + 12]
            rtmp = accb[:, DP + 12:DP + 13]
            opt6 = fwork[0:8, 462:468]
            nc.sync.dma_start(
                out=opt6,
                in_=opt_p.ap()[:, 0:3][:, None, :].to_broadcast([8, 2, 3]),
            )

            for w in range(2):
                # ------------- stage + column extraction -------------------
                coords_s = dramp.tile([8, COLN], F32, tag="coords_s")
                bonds_s = dramp.tile([8, COLN], F32, tag="bonds_s")
                angs_s = dramp.tile([8, COLN], F32, tag="angs_s")
                tors_s = dramp.tile([8, COLN], F32, tag="tors_s")
                col_dst = [coords_s, bonds_s, angs_s, tors_s]
                for s8 in range(8):
                    s = 8 * w + s8
                    stage = stagep.tile([128, LPP], F32, tag="stage")
                    nc.sync.dma_start(
                        out=stage[:], in_=feat.ap()[s].rearrange("(p f) -> p f", f=LPP)
                    )
                    stv = stage[:].rearrange("p (r n) -> p r n", n=9)
                    for k, col in enumerate((5, 6, 7, 8)):
                        cd = cextp.tile([128, CR], F32, tag="cd")
                        nc.vector.tensor_copy(out=cd[:], in_=stv[:, :, col])
                        nc.sync.dma_start(
                            out=col_dst[k][:][s8].rearrange("(p f) -> p f", f=CR),
                            in_=cd[:],
                        )

                # ------------- gather table (replicated coords) ------------
                table = tablep.tile([128, 3 * NATOMS], F32, tag="table")
                # partition p holds sample (p//16)'s coords; step-0 src AP
                # replicates each sample's row across its 16 partitions
                nc.sync.dma_start(
                    out=table[:],
                    in_=coords_s[:][:, None, 0:3 * NATOMS].to_broadcast(
                        [8, 16, 3 * NATOMS]),
                )

                # ------------- index readback + int16 conversion -----------
                iraw = idxrawp.tile([128, 3072], F32, tag="iraw")
                braw = iraw[:, 0:768]
                araw = iraw[:, 768:1792]
                trawi = iraw[:, 1792:3072]
                for s8 in range(8):
                    psl = iraw.rearrange("(a b) f -> a b f", b=16)[s8]
                    nc.sync.dma_start(
                        out=psl[:, 0:768],
                        in_=bonds_s[:][s8, 0:12288].rearrange("(j f) -> j f", j=16),
                    )
                    nc.sync.dma_start(
                        out=psl[:, 768:1792],
                        in_=angs_s[:][s8, 0:16384].rearrange("(j f) -> j f", j=16),
                    )
                    nc.sync.dma_start(
                        out=psl[:, 1792:3072],
                        in_=tors_s[:][s8, 0:20480].rearrange("(j f) -> j f", j=16),
                    )

                idxt = idx16p.tile([128, 12 * 256], I16, tag="idxt")

                def idx_list(n):
                    return idxt[:, 256 * n:256 * (n + 1)]

                bv = braw.rearrange("p (e k) -> p e k", k=3)
                av = araw.rearrange("p (e k) -> p e k", k=4)
                tv = trawi.rearrange("p (e k) -> p e k", k=5)
                for k in range(3):
                    nc.vector.tensor_copy(out=idx_list(k), in_=bv[:, :, k])
                for k in range(4):
                    nc.vector.tensor_copy(out=idx_list(3 + k), in_=av[:, :, k])
                for k in range(5):
                    nc.vector.tensor_copy(out=idx_list(7 + k), in_=tv[:, :, k])
                # lists: 0,1,2 = bond i,j,t; 3..6 = angle i,j,k,t; 7..11 = tor i,j,k,l,t

                def gather_dedup(idx_n, tab_ap, n_elems, d):
                    """ap_gather LIST indices; dedup 16x replicas via DRAM.
                    Returns dense [128, DP*d] tile (list position 256j+u on
                    partition 16c+j at cols u*d..)."""
                    ded = dedp.tile([8, LIST * d], F32, tag="ded")
                    for ch in range(NCHUNK):
                        g = gathp.tile([128, GCH * d], F32, tag="g")
                        nc.gpsimd.ap_gather(
                            out_ap=g[:].rearrange("p (n d) -> p n d", d=d),
                            in_ap=tab_ap,
                            idxs_ap=idx_list(idx_n)[:, (GCH // 16) * ch:(GCH // 16) * (ch + 1)],
                            channels=128,
                            num_elems=n_elems,
                            d=d,
                            num_idxs=GCH,
                        )
                        nc.sync.dma_start(
                            out=ded[:][:, GCH * d * ch:GCH * d * (ch + 1)],
                            in_=g[:].rearrange("(a b) f -> a b f", b=16)[:, 0, :],
                        )
                    dn = densep.tile([128, DP * d], F32, tag="dense")
                    ded_flat = ded[:].rearrange("s f -> (s f)")
                    nc.sync.dma_start(
                        out=dn[:],
                        in_=ded_flat.rearrange("(p f) -> p f", f=DP * d),
                    )
                    return dn

                tab3 = table[:].rearrange("p (n d) -> p n d", d=3)
                btab2 = btab.rearrange("p (n d) -> p n d", d=2)
                atab2 = atab.rearrange("p (n d) -> p n d", d=2)
                ttab4 = ttab.rearrange("p (n d) -> p n d", d=4)

                acc = acc6[:, 3 * w:3 * w + 3]
                nc.vector.memset(acc, 0.0)

                # ==================== BONDS ====================
                ci = gather_dedup(0, tab3, NATOMS, 3)
                cj = gather_dedup(1, tab3, NATOMS, 3)
                pb = gather_dedup(2, btab2, 15, 2)
                d3 = workp.tile([128, 3 * DP], F32, tag="w3a")
                nc.vector.tensor_sub(out=d3[:], in0=ci[:], in1=cj[:])
                d3s = workp.tile([128, 3 * DP], F32, tag="w3b")
                nc.vector.tensor_mul(out=d3s[:], in0=d3[:], in1=d3[:])
                wb = workp.tile([128, 8 * DP], F32, tag="w8")
                r2 = wb[:, 0:DP]
                nc.vector.tensor_reduce(
                    out=r2, in_=d3s[:].rearrange("p (n d) -> p n d", d=3),
                    axis=AX.X, op=Op.add,
                )
                r = wb[:, DP:2 * DP]
                nc.scalar.activation(r, r2, AF.Sqrt, bias=b_eps)
                pbv = pb[:].rearrange("p (n d) -> p n d", d=2)
                u = wb[:, 2 * DP:3 * DP]
                nc.vector.tensor_sub(out=u, in0=r, in1=pbv[:, :, 1])
                e = wb[:, 3 * DP:4 * DP]
                nc.scalar.activation(e, u, AF.Square)
                km = wb[:, 4 * DP:5 * DP]
                nc.vector.tensor_tensor(out=km, in0=pbv[:, :, 0], in1=mB, op=Op.mult)
                nc.vector.tensor_mul(out=scr, in0=e, in1=km)
                nc.vector.tensor_reduce(out=rtmp, in_=scr, axis=AX.X, op=Op.add)
                nc.vector.tensor_add(out=acc[:, 0:1], in0=acc[:, 0:1], in1=rtmp)

                # ==================== ANGLES ====================
                gi = gather_dedup(3, tab3, NATOMS, 3)
                gj = gather_dedup(4, tab3, NATOMS, 3)
                gk = gather_dedup(5, tab3, NATOMS, 3)
                pa = gather_dedup(6, atab2, 13, 2)
                v1 = workp.tile([128, 3 * DP], F32, tag="w3a")
                v2 = workp.tile([128, 3 * DP], F32, tag="w3b")
                nc.vector.tensor_sub(out=v1[:], in0=gi[:], in1=gj[:])
                nc.vector.tensor_sub(out=v2[:], in0=gk[:], in1=gj[:])
                prod = workp.tile([128, 3 * DP], F32, tag="w3c")
                wa = workp.tile([128, 8 * DP], F32, tag="w8")
                d11 = wa[:, 0:DP]
                d22 = wa[:, 1 * DP:2 * DP]
                d12 = wa[:, 2 * DP:3 * DP]

                def dot3(dst, a, b):
                    nc.vector.tensor_mul(out=prod[:], in0=a[:], in1=b[:])
                    nc.vector.tensor_reduce(
                        out=dst, in_=prod[:].rearrange("p (n d) -> p n d", d=3),
                        axis=AX.X, op=Op.add,
                    )

                dot3(d11, v1, v1)
                dot3(d22, v2, v2)
                dot3(d12, v1, v2)
                s1 = wa[:, 3 * DP:4 * DP]
                s2a = wa[:, 4 * DP:5 * DP]
                nc.scalar.activation(s1, d11, AF.Sqrt, bias=b_eps)
                nc.scalar.activation(s2a, d22, AF.Sqrt, bias=b_eps)
                den = wa[:, 5 * DP:6 * DP]
                nc.vector.tensor_mul(out=den, in0=s1, in1=s2a)
                cosv = wa[:, 6 * DP:7 * DP]
                nc.vector.reciprocal(out=den, in_=den)
                nc.vector.tensor_mul(out=cosv, in0=d12, in1=den)
                cosc = wa[:, 7 * DP:8 * DP]
                nc.vector.tensor_scalar(
                    out=cosc, in0=cosv, scalar1=-1.0 + 1e-6, scalar2=1.0 - 1e-6,
                    op0=Op.max, op1=Op.min,
                )
                # theta = arccos(cosc) via two bounded-arg arctan branches
                # (ACT Arctan domain is [-pi/2, pi/2] so |arg| <= 1 required):
                #  |c| >  s: theta = arctan(s/c) + pi*(c<0)
                #  |c| <= s: theta = pi/2 - arctan(c/s), s = sqrt(1-c^2)
                cc = wa[:, 0:DP]                       # d11 dead
                nc.scalar.activation(cc, cosc, AF.Square)
                om = wa[:, 1 * DP:2 * DP]              # d22 dead
                nc.vector.tensor_scalar(
                    out=om, in0=cc, scalar1=-1.0, scalar2=1.0, op0=Op.mult, op1=Op.add
                )
                sn = wa[:, 2 * DP:3 * DP]              # d12 dead
                nc.scalar.activation(sn, om, AF.Sqrt)
                sgn = wa[:, 3 * DP:4 * DP]             # s1 dead
                nc.vector.tensor_scalar(
                    out=sgn, in0=cosc, scalar1=0.0, scalar2=None, op0=Op.is_ge)
                nc.vector.tensor_scalar(
                    out=sgn, in0=sgn, scalar1=2e-18, scalar2=-1e-18,
                    op0=Op.mult, op1=Op.add)
                csafe = wa[:, 4 * DP:5 * DP]           # s2a dead
                nc.vector.tensor_add(out=csafe, in0=cosc, in1=sgn)
                ra = wa[:, 3 * DP:4 * DP]              # sgn dead
                nc.vector.reciprocal(out=csafe, in_=csafe)
                nc.vector.tensor_mul(out=ra, in0=sn, in1=csafe)
                nc.vector.tensor_scalar(
                    out=ra, in0=ra, scalar1=-1.0, scalar2=1.0, op0=Op.max, op1=Op.min)
                ata = wa[:, 4 * DP:5 * DP]             # csafe dead
                nc.scalar.activation(ata, ra, AF.Arctan)
                corr = wa[:, 5 * DP:6 * DP]            # den dead
                nc.vector.tensor_scalar(
                    out=corr, in0=cosc, scalar1=0.0, scalar2=PI, op0=Op.is_lt, op1=Op.mult
                )
                tha = wa[:, 3 * DP:4 * DP]             # ra dead
                nc.vector.tensor_add(out=tha, in0=ata, in1=corr)
                rb = wa[:, 4 * DP:5 * DP]              # ata dead
                nc.vector.reciprocal(out=sn, in_=sn)
                nc.vector.tensor_mul(out=rb, in0=cosc, in1=sn)
                nc.vector.tensor_scalar(
                    out=rb, in0=rb, scalar1=-1.0, scalar2=1.0, op0=Op.max, op1=Op.min)
                thb = wa[:, 5 * DP:6 * DP]             # corr dead
                nc.scalar.activation(thb, rb, AF.Arctan)
                nc.vector.tensor_scalar(
                    out=thb, in0=thb, scalar1=-1.0, scalar2=PI / 2.0,
                    op0=Op.mult, op1=Op.add)
                wi = workp.tile([128, 2 * DP], I32, tag="wi")
                mbr = wi[:, 0:DP]
                nc.vector.tensor_scalar(
                    out=mbr, in0=cc, scalar1=0.5, scalar2=None, op0=Op.is_gt)
                th = wa[:, 6 * DP:7 * DP]              # cosv dead
                nc.vector.select(out=th, mask=mbr, on_true=tha, on_false=thb)
                pav = pa[:].rearrange("p (n d) -> p n d", d=2)
                ua = wa[:, 0:DP]                       # cc dead
                nc.vector.tensor_sub(out=ua, in0=th, in1=pav[:, :, 1])
                ea = wa[:, 1 * DP:2 * DP]              # om dead
                nc.scalar.activation(ea, ua, AF.Square)
                kma = wa[:, 2 * DP:3 * DP]             # sn dead
                nc.vector.tensor_tensor(out=kma, in0=pav[:, :, 0], in1=mA, op=Op.mult)
                nc.vector.tensor_mul(out=scr, in0=ea, in1=kma)
                nc.vector.tensor_reduce(out=rtmp, in_=scr, axis=AX.X, op=Op.add)
                nc.vector.tensor_add(out=acc[:, 1:2], in0=acc[:, 1:2], in1=rtmp)

                # ==================== TORSIONS ====================
                ti = gather_dedup(7, tab3, NATOMS, 3)
                tj = gather_dedup(8, tab3, NATOMS, 3)
                tk_ = gather_dedup(9, tab3, NATOMS, 3)
                tl = gather_dedup(10, tab3, NATOMS, 3)
                pt = gather_dedup(11, ttab4, 25, 4)
                b1 = workp.tile([128, 3 * DP], F32, tag="w3a")
                b2 = workp.tile([128, 3 * DP], F32, tag="w3b")
                b3 = workp.tile([128, 3 * DP], F32, tag="w3c")
                nc.vector.tensor_sub(out=b1[:], in0=tj[:], in1=ti[:])
                nc.vector.tensor_sub(out=b2[:], in0=tk_[:], in1=tj[:])
                nc.vector.tensor_sub(out=b3[:], in0=tl[:], in1=tk_[:])
                pl = workp.tile([128, 9 * DP], F32, tag="w9")

                def plv(n):
                    return pl[:, DP * n:DP * (n + 1)]

                for m in range(3):
                    nc.vector.tensor_copy(
                        out=plv(0 + m),
                        in_=b1[:].rearrange("p (n d) -> p n d", d=3)[:, :, m])
                    nc.vector.tensor_copy(
                        out=plv(3 + m),
                        in_=b2[:].rearrange("p (n d) -> p n d", d=3)[:, :, m])
                    nc.vector.tensor_copy(
                        out=plv(6 + m),
                        in_=b3[:].rearrange("p (n d) -> p n d", d=3)[:, :, m])
                # n1 = b1 x b2 -> cr 0..2 ; n2 = b2 x b3 -> cr 3..5
                cr_ = workp.tile([128, 6 * DP], F32, tag="w6")

                def crv(n):
                    return cr_[:, DP * n:DP * (n + 1)]

                tmp = workp.tile([128, 2 * DP], F32, tag="w2")
                t0 = tmp[:, 0:DP]
                t1_ = tmp[:, DP:2 * DP]
                for m in range(3):
                    mp1, mp2 = (m + 1) % 3, (m + 2) % 3
                    nc.vector.tensor_mul(out=t0, in0=plv(0 + mp1), in1=plv(3 + mp2))
                    nc.vector.tensor_mul(out=t1_, in0=plv(0 + mp2), in1=plv(3 + mp1))
                    nc.vector.tensor_sub(out=crv(m), in0=t0, in1=t1_)
                    nc.vector.tensor_mul(out=t0, in0=plv(3 + mp1), in1=plv(6 + mp2))
                    nc.vector.tensor_mul(out=t1_, in0=plv(3 + mp2), in1=plv(6 + mp1))
                    nc.vector.tensor_sub(out=crv(3 + m), in0=t0, in1=t1_)
                wt = workp.tile([128, 8 * DP], F32, tag="w8")
                q2 = wt[:, 0:DP]
                nc.vector.tensor_mul(out=b1[:], in0=b2[:], in1=b2[:])  # b1 = scratch
                nc.vector.tensor_reduce(
                    out=q2, in_=b1[:].rearrange("p (n d) -> p n d", d=3),
                    axis=AX.X, op=Op.add,
                )
                # m1' = n1 x b2 (normalization folded into rn)
                mp = workp.tile([128, 3 * DP], F32, tag="w3a")

                def mpv(n):
                    return mp[:, DP * n:DP * (n + 1)]

                for m in range(3):
                    mp1, mp2 = (m + 1) % 3, (m + 2) % 3
                    nc.vector.tensor_mul(out=t0, in0=crv(mp1), in1=plv(3 + mp2))
                    nc.vector.tensor_mul(out=t1_, in0=crv(mp2), in1=plv(3 + mp1))
                    nc.vector.tensor_sub(out=mpv(m), in0=t0, in1=t1_)
                X = wt[:, 1 * DP:2 * DP]
                Y = wt[:, 2 * DP:3 * DP]
                nc.vector.tensor_mul(out=t0, in0=crv(0), in1=crv(3))
                nc.vector.tensor_mul(out=t1_, in0=crv(1), in1=crv(4))
                nc.vector.tensor_add(out=X, in0=t0, in1=t1_)
                nc.vector.tensor_mul(out=t0, in0=crv(2), in1=crv(5))
                nc.vector.tensor_add(out=X, in0=X, in1=t0)
                nc.vector.tensor_mul(out=t0, in0=mpv(0), in1=crv(3))
                nc.vector.tensor_mul(out=t1_, in0=mpv(1), in1=crv(4))
                nc.vector.tensor_add(out=Y, in0=t0, in1=t1_)
                nc.vector.tensor_mul(out=t0, in0=mpv(2), in1=crv(5))
                nc.vector.tensor_add(out=Y, in0=Y, in1=t0)
                rn = wt[:, 3 * DP:4 * DP]
                nc.scalar.activation(rn, q2, AF.Sqrt, bias=b_eps)
                y = wt[:, 4 * DP:5 * DP]
                nc.vector.reciprocal(out=rn, in_=rn)
                nc.vector.tensor_mul(out=y, in0=Y, in1=rn)
                hx = wt[:, 5 * DP:6 * DP]
                hy = wt[:, 6 * DP:7 * DP]
                nc.scalar.activation(hx, X, AF.Square)
                nc.scalar.activation(hy, y, AF.Square)
                h = wt[:, 7 * DP:8 * DP]
                nc.vector.tensor_add(out=h, in0=hx, in1=hy)
                rh = wt[:, 5 * DP:6 * DP]              # hx dead
                nc.scalar.activation(rh, h, AF.Sqrt, bias=b_tiny)
                c = wt[:, 0:DP]                        # q2 dead
                s = wt[:, 6 * DP:7 * DP]               # hy dead
                nc.vector.reciprocal(out=rh, in_=rh)
                nc.vector.tensor_mul(out=c, in0=X, in1=rh)
                nc.vector.tensor_mul(out=s, in0=y, in1=rh)
                # Chebyshev: cos/sin of 2phi and 3phi (reuse pl slices: b1/b3
                # component planes are dead after the cross products)
                cc_ = plv(0)
                c2 = plv(1)
                s2 = plv(2)
                c3 = plv(6)
                s3 = plv(7)
                sc = plv(8)
                nc.scalar.activation(cc_, c, AF.Square)
                nc.vector.tensor_scalar(
                    out=c2, in0=cc_, scalar1=2.0, scalar2=-1.0, op0=Op.mult, op1=Op.add)
                nc.vector.tensor_mul(out=sc, in0=s, in1=c)
                nc.vector.tensor_scalar(
                    out=s2, in0=sc, scalar1=2.0, scalar2=None, op0=Op.mult)
                nc.vector.tensor_scalar(
                    out=t0, in0=cc_, scalar1=4.0, scalar2=-3.0, op0=Op.mult, op1=Op.add)
                nc.vector.tensor_mul(out=c3, in0=t0, in1=c)
                nc.vector.tensor_scalar(
                    out=t0, in0=cc_, scalar1=4.0, scalar2=-1.0, op0=Op.mult, op1=Op.add)
                nc.vector.tensor_mul(out=s3, in0=t0, in1=s)
                ptv = pt[:].rearrange("p (n d) -> p n d", d=4)
                wi2 = workp.tile([128, 2 * DP], I32, tag="wi")
                m2m = wi2[:, 0:DP]
                m3m = wi2[:, DP:2 * DP]
                nc.vector.tensor_scalar(
                    out=m2m, in0=ptv[:, :, 3], scalar1=2.0, scalar2=None, op0=Op.is_equal)
                nc.vector.tensor_scalar(
                    out=m3m, in0=ptv[:, :, 3], scalar1=3.0, scalar2=None, op0=Op.is_equal)
                cn = wt[:, 3 * DP:4 * DP]              # rn dead
                sn2 = wt[:, 4 * DP:5 * DP]             # y dead
                nc.vector.select(out=cn, mask=m2m, on_true=c2, on_false=c)
                nc.vector.select(out=cn, mask=m3m, on_true=c3, on_false=cn)
                nc.vector.select(out=sn2, mask=m2m, on_true=s2, on_false=s)
                nc.vector.select(out=sn2, mask=m3m, on_true=s3, on_false=sn2)
                tt1 = wt[:, 5 * DP:6 * DP]             # rh dead
                tt2 = wt[:, 6 * DP:7 * DP]             # s dead (selects done)
                nc.vector.tensor_mul(out=tt1, in0=cn, in1=ptv[:, :, 1])
                nc.vector.tensor_mul(out=tt2, in0=sn2, in1=ptv[:, :, 2])
                esum = wt[:, 7 * DP:8 * DP]            # h dead
                nc.vector.tensor_add(out=esum, in0=tt1, in1=tt2)
                nc.vector.tensor_scalar(
                    out=esum, in0=esum, scalar1=1.0, scalar2=None, op0=Op.add)
                kmt = wt[:, 0:DP]                      # c dead
                nc.vector.tensor_tensor(out=kmt, in0=ptv[:, :, 0], in1=mT, op=Op.mult)
                nc.vector.tensor_mul(out=scr, in0=esum, in1=kmt)
                nc.vector.tensor_reduce(out=rtmp, in_=scr, axis=AX.X, op=Op.add)
                nc.vector.tensor_add(out=acc[:, 2:3], in0=acc[:, 2:3], in1=rtmp)

            # ------------- final reduction: [128, 6] -> [8, 6] -> out ------
            pacc = psump.tile([8, 6], F32, tag="pacc")
            nc.tensor.matmul(out=pacc[:], lhsT=blk, rhs=acc6, start=True, stop=True)
            nc.vector.tensor_copy(out=otmp, in_=pacc[:])
            nc.vector.tensor_mul(out=otmp, in0=otmp, in1=opt6)
            nc.sync.dma_start(out=out_d.ap()[0:8, :], in_=otmp[:, 0:3])
            nc.sync.dma_start(out=out_d.ap()[8:16, :], in_=otmp[:, 3:6])

    nc.compile()
    return nc


@functools.lru_cache(maxsize=1)
def _get_nc():
    return build_nc()


def make_in_maps(inputs):
    """Shard full inputs into 8 per-core input maps."""
    feats = np.ascontiguousarray(inputs["features"], dtype=np.float32)
    Bf = feats.shape[0]
    flat = feats.reshape(Bf, -1)
    flat = np.concatenate(
        [flat, np.zeros((Bf, FLATPAD - flat.shape[1]), np.float32)], axis=1
    )
    bond_type = np.ascontiguousarray(inputs["bond_type"], np.float32)
    angle_type = np.ascontiguousarray(inputs["angle_type"], np.float32)
    tor_type = np.ascontiguousarray(inputs["tor_type"], np.float32)
    mult_f = np.ascontiguousarray(inputs["multiplicity"], np.float32).reshape(1, 25)
    opt = np.ascontiguousarray(inputs["opt_pars"], np.float32).reshape(1, 47)
    n_nc = Bf // NS
    in_maps = []
    for k in range(n_nc):
        in_maps.append({
            "features": flat[NS * k:NS * (k + 1)],
            "bond_type": bond_type,
            "angle_type": angle_type,
            "tor_type": tor_type,
            "mult_f": mult_f,
            "opt_pars": opt,
        })
    return in_maps


def kernel(**inputs) -> np.ndarray:
    from concourse.bass_utils import run_bass_kernel_spmd

    nc = _get_nc()
    in_maps = make_in_maps(inputs)
    res = run_bass_kernel_spmd(nc, in_maps, core_ids=list(range(len(in_maps))))
    outs = [res.results[k]["out"] for k in range(len(in_maps))]
    return np.concatenate(outs, axis=0).astype(np.float32)


def simulate_one_core(inputs, nc=None):
    """CoreSim a single NC on the first 16 samples (for correctness dev)."""
    import concourse.bass_interp as bass_interp

    if nc is None:
        nc = _get_nc()
    in_map = make_in_maps(inputs)[0]
    sim = bass_interp.MultiCoreSim(nc, 1)
    for name, val in in_map.items():
        sim.cores[0].tensor(name)[:] = val
    sim.simulate(check_with_hw=False)
    return np.array(sim.cores[0].mem_tensor("out"))


if __name__ == "__main__":
    nc = build_nc()
    print("build ok")

